# revision 3
# baseline (speedup 1.0000x reference)
"""Trainium2 Bass kernel v2: DyT-prenorm attention (RoPE, causal+mask) +
top-2-of-16 MoE with a shared expert.

Differences vs v1 baseline:
  * Attention computes scores TRANSPOSED (scT[k,q] = krot^T qrot), so the
    exp output feeds the P@V matmul directly as the moving operand -- no
    per-block PE transposes / DVE copies.  Softmax skips the max-subtract
    (scores are O(1) for this problem) and the denominator comes free from
    a 65th all-ones column appended to V.  Normalization is applied once at
    the end via a rank-1 (ones x 1/sum) PE broadcast.
  * Experts (ek/ev), shared-expert up (sk) and their activations run in
    fp8e4m3 with DoubleRow matmuls (2 contraction chunks per MM).  Scales
    are folded host-side: ek*32, sk*32, h2*4 (via g2/b2*4), gelu scale
    1/128, gates*16, ev*16, shared sv(bf16)*256, final copy *1/256.
  * A tiny barrier AllGather is issued at t=0 (overlaps the weight loads)
    to absorb inter-core launch skew / first-collective setup cost.
  * Dummy PE matmuls keep the tensor engine HAM-warm through the mid-kernel
    AllGather stall; tiny dummy activations prefetch the ACT function
    tables (tanh/exp/gelu) before the real uses.
"""

import os
import numpy as np
import ml_dtypes

BF = ml_dtypes.bfloat16
F8 = ml_dtypes.float8_e4m3

S = 512      # tokens (B=1)
Dm = 512     # d_model
H = 8        # heads
HD = 64      # head dim
E = 16       # experts
FF = 512     # expert hidden
P = 128
NCORES = 8
DC = Dm // P    # 4 d-model chunks
TCH = S // P    # 4 token chunks
FCH = FF // P   # 4 ff chunks
SHF = FF // NCORES  # shared-expert ff slice per core (64)
DSH = Dm // NCORES  # output row shard per core (64)

# fp8 scale folding
SH = 4.0      # h2 fp8 scale (folded into g2/b2)
SWE = 32.0    # ek/sk fp8 scale
SV = 16.0     # ev fp8 scale
SG = 16.0     # gate scale (folded into rep16)
GELU_SC = 1.0 / (SH * SWE)    # up-projection descale before gelu
FIN_SC = 1.0 / (SG * SV)      # final moe descale
SVSH = SG * SV                # shared-expert sv (bf16) pre-scale

GELU_C = float(2.0 * np.sqrt(2.0 / np.pi))
GELU_A = 0.044715

_PROG_CACHE = {}
LAST_INFO = {}


def _layouts(compact_mask):
    """Column layouts of the packed constant arrays (shared host/device)."""
    def lay(blocks):
        off, out = 0, {}
        for name, cols in blocks:
            out[name] = (off, cols)
            off += cols
        return out, off

    atn, atn_c = lay([
        ("cd", S), ("cs", S),
        ("maskT", P if compact_mask else TCH * S), ("idbf", P), ("ones", P),
        ("c64", HD),
    ])
    at8, at8_c = lay([
        ("wqk", DC * P), ("wqs", DC * P), ("wv", DC * HD), ("gw8", DC * E),
    ])
    mid, mid_c = lay([("wo", DC * Dm)])
    mo8, mo8_c = lay([
        ("ek", 2 * DC * FF), ("ev", 2 * FCH * Dm), ("sk", DC * SHF),
    ])
    m16, m16_c = lay([("sv", Dm)])
    p32, p32_c = lay([
        ("g1s", DC), ("b1s", DC), ("g2", DC), ("b2", DC),
        ("g2s", DC), ("b2s", DC),
        ("gb", TCH * E), ("selm", DC * DSH),
        ("idf", P),
    ])
    return (atn, atn_c), (at8, at8_c), (mid, mid_c), (mo8, mo8_c), \
        (m16, m16_c), (p32, p32_c)


def _build_program(a1v: float, a2v: float, sim_gelu: bool,
                   compact_mask: bool, rs8: bool):
    import concourse.bass as bass
    import concourse.mybir as mybir
    import concourse.tile as tile
    from concourse import bacc

    f32 = mybir.dt.float32
    bf16 = mybir.dt.bfloat16
    fp8 = mybir.dt.float8e4
    Alu = mybir.AluOpType
    Act = mybir.ActivationFunctionType
    PM = mybir.MatmulPerfMode
    AX = mybir.AxisListType
    ts = bass.ts

    (atn_l, atn_c), (at8_l, at8_c), (mid_l, mid_c), (mo8_l, mo8_c), \
        (m16_l, m16_c), (p32_l, p32_c) = _layouts(compact_mask)

    nc = bacc.Bacc(
        "TRN2", target_bir_lowering=False, debug=False, num_devices=NCORES
    )

    def inp(name, shape, dt=f32):
        return nc.dram_tensor(name, list(shape), dt, kind="ExternalInput").ap()

    xT_d = inp("xT", (P, DC * S))
    atn_d = inp("atn16", (P, atn_c), bf16)
    at8_d = inp("atn8", (P, at8_c), fp8)
    mid_d = inp("mid8", (P, mid_c), fp8)
    mo8_d = inp("moe8", (P, mo8_c), fp8)
    m16_d = inp("moe16", (P, m16_c), bf16)
    p32_d = inp("p32", (P, p32_c))

    outT_d = nc.dram_tensor("outT", [DSH, S], f32, kind="ExternalOutput").ap()

    with tile.TileContext(nc, num_cores=NCORES) as tc:
        with (
            tc.tile_pool(name="cst", bufs=1) as cst,
            tc.tile_pool(name="tmp", bufs=3) as tmp,
            tc.tile_pool(name="ps", bufs=2, space="PSUM") as psp,
            tc.tile_pool(name="dram", bufs=1, space="DRAM") as drp,
        ):
            # ---------- packed loads (7 DMA dispatches, 2 queues) ----------
            xTt = cst.tile((P, DC * S), f32, name="xTt", tag="xTt")
            nc.sync.dma_start(xTt[:, 0:2 * S], xT_d[:, 0:2 * S])
            pk32 = cst.tile((P, p32_c), f32, name="pk32", tag="pk32")
            nc.scalar.dma_start(pk32[:], p32_d[:])
            at8 = cst.tile((P, at8_c), fp8, name="at8", tag="at8")
            nc.sync.dma_start(at8[:], at8_d[:])
            atn16 = cst.tile((P, atn_c), bf16, name="atn16", tag="atn16")
            nc.sync.dma_start(atn16[:], atn_d[:])
            nc.sync.dma_start(xTt[:, 2 * S:4 * S], xT_d[:, 2 * S:4 * S])
            mid8 = cst.tile((P, mid_c), fp8, name="mid8", tag="mid8")
            nc.scalar.dma_start(mid8[:], mid_d[:])
            moe8 = cst.tile((P, mo8_c), fp8, name="moe8", tag="moe8")
            nc.sync.dma_start(moe8[:], mo8_d[:])
            m16t = cst.tile((P, m16_c), bf16, name="m16t", tag="m16t")
            nc.scalar.dma_start(m16t[:], m16_d[:])

            def asl(name, c=0, w=None):  # attention-pack slice
                off, cols = atn_l[name]
                w = cols if w is None else w
                return atn16[:, off + c * w: off + (c + 1) * w]

            def a8l(name, c=0, w=None):  # fp8 attention-pack slice
                off, cols = at8_l[name]
                w = cols if w is None else w
                return at8[:, off + c * w: off + (c + 1) * w]

            def psl(name, c=0, w=None):  # fp32-pack slice
                off, cols = p32_l[name]
                w = cols if w is None else w
                return pk32[:, off + c * w: off + (c + 1) * w]

            idbf = asl("idbf")
            ones16 = asl("ones")
            idf = psl("idf")
            # 3D chunk-major views of the fp8 weight packs
            ek8 = moe8[:, mo8_l["ek"][0]: mo8_l["ek"][0] + 2 * DC * FF]
            ek8 = ek8.rearrange("p (c f) -> p c f", c=2 * DC)
            ev8 = moe8[:, mo8_l["ev"][0]: mo8_l["ev"][0] + 2 * FCH * Dm]
            ev8 = ev8.rearrange("p (c f) -> p c f", c=2 * FCH)
            sk8 = moe8[:, mo8_l["sk"][0]: mo8_l["sk"][0] + DC * SHF]
            sk8 = sk8.rearrange("p (c f) -> p c f", c=DC)
            wqk8 = a8l("wqk").rearrange("p (c f) -> p c f", c=DC)
            wqs8 = a8l("wqs").rearrange("p (c f) -> p c f", c=DC)
            wv8 = a8l("wv").rearrange("p (c f) -> p c f", c=DC)
            wo8 = mid8[:, mid_l["wo"][0]: mid_l["wo"][0] + DC * Dm]
            wo8 = wo8.rearrange("p (c f) -> p c f", c=DC)

            dum = tmp.tile((1, 8), f32, name="dum", tag="dum", bufs=1)

            # ---------- phase 1: dyt1 + QK/V ----------
            hT8 = cst.tile((P, DC, S), fp8, name="hT8", tag="hT8")
            for c in range(DC):
                th = tmp.tile((P, S), f32, name="th", tag="t32")
                nc.scalar.activation(th[:], xTt[:, ts(c, S)], Act.Tanh,
                                     scale=float(a1v))
                nc.vector.scalar_tensor_tensor(
                    hT8[:, c, :], th[:], psl("g1s", c, 1),
                    psl("b1s", c, 1).to_broadcast((P, S)),
                    op0=Alu.mult, op1=Alu.add,
                )
            # prefetch the Exp table while QK/V matmuls run
            nc.scalar.activation(dum[:], pk32[0:1, 0:8], Act.Exp)

            # qkT = [wq*0.125 | wk]^T @ h  -> [128 (q64|k64), S], plus the
            # same with per-head halves pre-swapped (for rope's rotate-half)
            qk_ps = psp.tile((P, S), f32, name="qk_ps", tag="mm")
            for j in range(2):
                nc.tensor.matmul(
                    qk_ps[:], lhsT=wqk8[:, 2 * j:2 * j + 2, :],
                    rhs=hT8[:, 2 * j:2 * j + 2, :],
                    start=(j == 0), stop=(j == 1), perf_mode=PM.DoubleRow,
                )
            qs_ps = psp.tile((P, S), f32, name="qs_ps", tag="mm")
            for j in range(2):
                nc.tensor.matmul(
                    qs_ps[:], lhsT=wqs8[:, 2 * j:2 * j + 2, :],
                    rhs=hT8[:, 2 * j:2 * j + 2, :],
                    start=(j == 0), stop=(j == 1), perf_mode=PM.DoubleRow,
                )

            # v (token-major) in one PSUM tile [128, 4, 64]; SBUF copy gets
            # a 65th all-ones column (the softmax-denominator trick).
            v_ps = psp.tile((P, TCH, HD), f32, name="v_ps", tag="avT",
                            bufs=1)
            for t in range(TCH):
                for j in range(2):
                    nc.tensor.matmul(
                        v_ps[:, t, :], lhsT=hT8[:, 2 * j:2 * j + 2, ts(t, P)],
                        rhs=wv8[:, 2 * j:2 * j + 2, :],
                        start=(j == 0), stop=(j == 1), perf_mode=PM.DoubleRow,
                    )
            v16 = []
            for t in range(TCH):
                vt = cst.tile((P, HD + 1), bf16, name=f"v16_{t}",
                              tag=f"v16_{t}")
                nc.any.tensor_scalar(vt[:, 0:HD], v_ps[:, t, :],
                                     1.0 / 128.0, None, op0=Alu.mult)
                nc.vector.memset(vt[:, HD:HD + 1], 1.0)
                v16.append(vt)

            # rope: qkrot = qk*cos + swapped_qk*(+-sin), 3 full-width DVE ops
            r1 = tmp.tile((P, S), f32, name="r1", tag="t32")
            nc.vector.tensor_tensor(r1[:], qk_ps[:], asl("cd"), Alu.mult)
            sw = tmp.tile((P, S), f32, name="sw", tag="t32")
            nc.vector.tensor_tensor(sw[:], qs_ps[:], asl("cs"), Alu.mult)
            qrot_t = cst.tile((HD, S), bf16, name="qrot", tag="qrot")
            nc.vector.tensor_tensor(qrot_t[:], r1[0:HD, :], sw[0:HD, :],
                                    Alu.add)
            krot_t = cst.tile((HD, S), bf16, name="krot", tag="krot")
            nc.vector.tensor_tensor(krot_t[:], r1[HD:P, :], sw[HD:P, :],
                                    Alu.add)
            qrot = qrot_t[:]
            krot = krot_t[:]

            # ---------- transposed scores + unnormalized softmax + A@V ----
            # avT_ps rows 0:64 = V^T @ exp(S^T); row 64 = colsums (denom).
            avT_ps = psp.tile((HD + 1, S), f32, name="avT_ps", tag="avT",
                              bufs=1)
            uT16 = []
            for j in range(TCH):
                L = S - j * P
                scT_ps = psp.tile((P, S), f32, name="scT_ps", tag="mm")
                nc.tensor.matmul(
                    scT_ps[:, :L], lhsT=krot[:, ts(j, P)],
                    rhs=qrot[:, j * P:S], start=True, stop=True,
                )
                u = tmp.tile((P, S), bf16, name=f"uT_{j}", tag=f"uT_{j}",
                             bufs=1)
                nc.scalar.activation(u[:, :L], scT_ps[:, :L], Act.Exp)
                # masking as a 0/1 multiply on the exp output (GPSIMD can't
                # touch PSUM); exp(s)*m == exp(s + log m) for m in {0,1}
                if compact_mask:
                    # only the diagonal block needs masking (identical for
                    # every j); off-diag causal blocks are fully visible
                    nc.any.tensor_tensor(
                        u[:, :P], u[:, :P], asl("maskT"), Alu.mult,
                    )
                else:
                    nc.any.tensor_tensor(
                        u[:, :L], u[:, :L],
                        asl("maskT", j, S)[:, :L], Alu.mult,
                    )
                uT16.append(u)
                # query-region j is complete once exp_j exists: accumulate
                # its k-chunks jj=0..j as one contiguous PSUM group
                for jj in range(j + 1):
                    nc.tensor.matmul(
                        avT_ps[:, ts(j, P)], lhsT=v16[jj][:],
                        rhs=uT16[jj][:, (j - jj) * P:(j - jj + 1) * P],
                        start=(jj == 0), stop=(jj == j),
                    )
            # prefetch the Tanh table (dyt2) while the AllGather runs
            nc.scalar.activation(dum[:], pk32[0:1, 0:8], Act.Tanh)

            # normalization: rinv = 1/colsum; rank-1 broadcast over 64 rows
            rinv16 = cst.tile((1, S), bf16, name="rinv16", tag="rinv16")
            with nc.allow_low_precision(reason="softmax denom, bf16 is fine"):
                nc.vector.reciprocal(rinv16[:], avT_ps[HD:HD + 1, :])
            # rep = 64/sum (the 64 pre-scales ao for fp8; descaled after wo)
            rep_ps = psp.tile((HD, S), f32, name="rep_ps", tag="lg", bufs=1)
            nc.tensor.matmul(rep_ps[:], lhsT=asl("c64")[0:1, :],
                             rhs=rinv16[:], start=True, stop=True)
            rep_s = tmp.tile((HD, S), bf16, name="rep_s", tag="rep_s",
                             bufs=1)
            nc.vector.tensor_copy(rep_s[:], rep_ps[:])
            ao16 = cst.tile((HD, S), fp8, name="ao16", tag="ao16")
            nc.vector.tensor_tensor(ao16[:], avT_ps[0:HD, :], rep_s[:],
                                    Alu.mult)

            # ---------- AllGather attention outputs (heads, fp8) ----------
            ag_in = drp.tile((HD, S), fp8, name="ag_in")
            ag_out = drp.tile((H * HD, S), fp8, name="ag_out",
                              addr_space="Shared")
            nc.sync.dma_start(ag_in[:], ao16[:])
            nc.gpsimd.collective_compute(
                "AllGather", Alu.bypass,
                replica_groups=[list(range(NCORES))],
                ins=[ag_in[:]], outs=[ag_out[:]],
            )
            aoT = cst.tile((P, DC, S), fp8, name="aoT", tag="aoT")
            nc.sync.dma_start(
                aoT[:], ag_out[:].rearrange("(c p) s -> p c s", p=P))

            # ---------- wo projection + residual + dyt2 ----------
            h28 = cst.tile((P, DC, S), fp8, name="h28", tag="h28")
            x1T32 = []
            for m in range(DC):
                pw = psp.tile((P, S), f32, name="pw", tag="mm")
                for j in range(2):
                    nc.tensor.matmul(
                        pw[:], lhsT=wo8[:, 2 * j:2 * j + 2, m * P:(m + 1) * P],
                        rhs=aoT[:, 2 * j:2 * j + 2, :],
                        start=(j == 0), stop=(j == 1),
                        perf_mode=PM.DoubleRow,
                    )
                x1 = cst.tile((P, S), f32, name=f"x1T{m}", tag=f"x1T{m}")
                nc.vector.scalar_tensor_tensor(
                    x1[:], pw[:], 1.0 / 2048.0, xTt[:, ts(m, S)],
                    op0=Alu.mult, op1=Alu.add,
                )
                x1T32.append(x1)
                th = tmp.tile((P, S), f32, name="th2", tag="t32")
                nc.scalar.activation(th[:], x1[:], Act.Tanh, scale=float(a2v))
                nc.vector.scalar_tensor_tensor(
                    h28[:, m, :], th[:], psl("g2s", m, 1),
                    psl("b2s", m, 1).to_broadcast((P, S)),
                    op0=Alu.mult, op1=Alu.add,
                )
            # prefetch the Exp table (router softmax)
            nc.scalar.activation(dum[:], pk32[0:1, 0:8], Act.Exp)

            # selected x1 rows [64c, 64c+64): exact one-hot fp32 matmuls
            # (keeps the selection off the busy DVE; PSUM tile lives to the
            # final residual add)
            xsel_ps = psp.tile((DSH, S), f32, name="xsel_ps", tag="avT",
                               bufs=1)
            for m in range(DC):
                nc.tensor.matmul(
                    xsel_ps[:], lhsT=psl("selm", m, DSH), rhs=x1T32[m][:],
                    start=(m == 0), stop=(m == DC - 1),
                )

            # ---------- router (fp32) ----------
            lg_ps = psp.tile((P, TCH, E), f32, name="lg_ps", tag="lg", bufs=1)
            gw8v = a8l("gw8").rearrange("p (c e) -> p c e", c=DC)
            for t in range(TCH):
                for j in range(2):
                    nc.tensor.matmul(
                        lg_ps[:, t, :], lhsT=h28[:, 2 * j:2 * j + 2, ts(t, P)],
                        rhs=gw8v[:, 2 * j:2 * j + 2, :],
                        start=(j == 0), stop=(j == 1),
                        perf_mode=PM.DoubleRow,
                    )

            # ---------- expert up-projections (fp8 DoubleRow) ----------
            def gelu_scaled(dst_rows, src_ps, rows):
                """gelu_tanh(GELU_SC * src_ps[:rows]) -> bf16 tile."""
                g0 = tmp.tile((P, S), bf16, name="g0", tag="g16", bufs=10)
                if not sim_gelu:
                    nc.scalar.activation(g0[:rows], src_ps[:rows],
                                         Act.Gelu_apprx_tanh, scale=GELU_SC)
                    return g0
                u16 = tmp.tile((P, S), bf16, name="u16", tag="u16", bufs=3)
                nc.vector.tensor_scalar(u16[:rows], src_ps[:rows], GELU_SC,
                                        None, op0=Alu.mult)
                x2 = tmp.tile((P, S), bf16, name="x2", tag="x2", bufs=3)
                nc.vector.tensor_tensor(x2[:rows], u16[:rows], u16[:rows],
                                        Alu.mult)
                t1 = tmp.tile((P, S), bf16, name="t1", tag="x2", bufs=3)
                nc.vector.tensor_scalar(
                    t1[:rows], x2[:rows], GELU_A, 1.0,
                    op0=Alu.mult, op1=Alu.add,
                )
                mm_ = tmp.tile((P, S), bf16, name="mm_", tag="x2", bufs=3)
                nc.vector.tensor_tensor(mm_[:rows], u16[:rows], t1[:rows],
                                        Alu.mult)
                sg = tmp.tile((P, S), bf16, name="sg", tag="x2", bufs=3)
                nc.scalar.activation(sg[:rows], mm_[:rows], Act.Sigmoid,
                                     scale=GELU_C)
                nc.vector.tensor_tensor(g0[:rows], u16[:rows], sg[:rows],
                                        Alu.mult)
                return g0

            g0s = [[], []]
            for el in range(2):
                for fc in range(FCH):
                    up_ps = psp.tile((P, S), f32, name="up_ps", tag="mm")
                    for j in range(2):
                        nc.tensor.matmul(
                            up_ps[:],
                            lhsT=ek8[:, el * DC + 2 * j: el * DC + 2 * j + 2,
                                     fc * P:(fc + 1) * P],
                            rhs=h28[:, 2 * j:2 * j + 2, :],
                            start=(j == 0), stop=(j == 1),
                            perf_mode=PM.DoubleRow,
                        )
                    g0s[el].append(gelu_scaled(None, up_ps, P))
            # shared-expert up (fp8 DoubleRow)
            su_ps2 = psp.tile((SHF, S), f32, name="su_ps2", tag="mm")
            for j in range(2):
                nc.tensor.matmul(
                    su_ps2[:], lhsT=sk8[:, 2 * j:2 * j + 2, :],
                    rhs=h28[:, 2 * j:2 * j + 2, :],
                    start=(j == 0), stop=(j == 1),
                    perf_mode=PM.DoubleRow,
                )
            gs16 = gelu_scaled(None, su_ps2, SHF)

            # ---------- top-2 gates (DVE, overlaps the ups) ----------
            gb_ap = psl("gb").rearrange("p (t e) -> p t e", e=E)
            lg32 = cst.tile((P, TCH, E), f32, name="lg32", tag="lg32")
            nc.vector.scalar_tensor_tensor(
                lg32[:], lg_ps[:], 1.0 / 128.0, gb_ap,
                op0=Alu.mult, op1=Alu.add,
            )
            ex32 = cst.tile((P, TCH, E), f32, name="ex32", tag="ex32")
            nc.scalar.activation(ex32[:], lg32[:], Act.Exp, scale=1.0)
            ssum4 = cst.tile((P, TCH), f32, name="ssum4", tag="ssum4")
            nc.vector.reduce_sum(ssum4[:], ex32[:], axis=AX.X)
            rinv4 = cst.tile((P, TCH), f32, name="rinv4", tag="rinv4")
            nc.vector.reciprocal(rinv4[:], ssum4[:])
            # prefetch the Gelu table while the top-2 chain runs
            if not sim_gelu:
                nc.scalar.activation(dum[:], pk32[0:1, 0:8],
                                     Act.Gelu_apprx_tanh)
            prb = cst.tile((P, TCH, E), f32, name="prb", tag="prb")
            nc.vector.tensor_tensor(
                prb[:], ex32[:], rinv4[:, :, None].to_broadcast((P, TCH, E)),
                Alu.mult,
            )
            m1 = cst.tile((P, TCH), f32, name="m1", tag="m1")
            nc.vector.reduce_max(m1[:], prb[:], axis=AX.X)
            ge1 = cst.tile((P, TCH, E), f32, name="ge1", tag="ge1")
            nc.vector.tensor_tensor(
                ge1[:], prb[:], m1[:, :, None].to_broadcast((P, TCH, E)),
                Alu.is_ge,
            )
            msk = cst.tile((P, TCH, E), f32, name="msk", tag="msk")
            nc.vector.scalar_tensor_tensor(
                msk[:], ge1[:], -1e9, prb[:], op0=Alu.mult, op1=Alu.add
            )
            m2 = cst.tile((P, TCH), f32, name="m2", tag="m2")
            nc.vector.reduce_max(m2[:], msk[:], axis=AX.X)
            ge2 = cst.tile((P, TCH, E), f32, name="ge2", tag="ge2")
            nc.vector.tensor_tensor(
                ge2[:], prb[:], m2[:, :, None].to_broadcast((P, TCH, E)),
                Alu.is_ge,
            )
            wg = cst.tile((P, TCH, E), f32, name="wg", tag="wg")
            nc.vector.tensor_tensor(wg[:], prb[:], ge2[:], Alu.mult)

            # transpose the two local experts' gate columns to rows
            wrow = [
                cst.tile((1, S), bf16, name=f"wrow{el}", tag=f"wrow{el}")
                for el in range(2)
            ]
            for t in range(TCH):
                for el in range(2):
                    wt_ps = psp.tile((1, P), f32, name="wt_ps",
                                     tag="lg", bufs=1)
                    nc.tensor.transpose(wt_ps[:], wg[:, t, el:el + 1], idf)
                    nc.vector.tensor_copy(wrow[el][:, ts(t, P)], wt_ps[:])

            # broadcast local-expert gate rows across partitions (rank-1 mm),
            # scaled by SG for the fp8 down-projection.
            rep16 = []
            for el in range(2):
                rp_ps = psp.tile((P, S), f32, name="rp_ps", tag="mm")
                nc.tensor.matmul(
                    rp_ps[:], lhsT=ones16[0:1, :], rhs=wrow[el][:],
                    start=True, stop=True,
                )
                rp = cst.tile((P, S), bf16, name=f"rep16_{el}",
                              tag=f"rep16_{el}")
                nc.vector.tensor_scalar(rp[:], rp_ps[:], float(SG), None,
                                        op0=Alu.mult)
                rep16.append(rp)

            # ---------- gate the gelu outputs (fp8), down-projections -----
            g8 = cst.tile((P, 2 * FCH, S), fp8, name="g8", tag="g8")
            for el in range(2):
                for fc in range(FCH):
                    nc.gpsimd.tensor_tensor(
                        g8[:, el * FCH + fc, :], g0s[el][fc][:], rep16[el][:],
                        Alu.mult,
                    )
            moe_ps = [
                psp.tile((P, S), f32, name=f"moe_ps{m}", tag="moe", bufs=4)
                for m in range(DC)
            ]
            # fp8 RS halves the collective bytes; payload pre-scaled by
            # rs_isc so fp8 keeps ~2 significant digits of the partials
            rs_dt = fp8 if rs8 else bf16
            rs_isc = 16.0 if rs8 else 1.0
            rs_fsc = FIN_SC * rs_isc
            rs_in = drp.tile((Dm, S), rs_dt, name="rs_in")
            fin = tmp.tile((P, DC, S), rs_dt, name="fin", tag="fin", bufs=1)
            for m in range(DC):
                for el in range(2):
                    for j in range(2):
                        nc.tensor.matmul(
                            moe_ps[m][:],
                            lhsT=ev8[:, el * FCH + 2 * j: el * FCH + 2 * j + 2,
                                     m * P:(m + 1) * P],
                            rhs=g8[:, el * FCH + 2 * j: el * FCH + 2 * j + 2, :],
                            start=(el == 0 and j == 0), stop=False,
                            perf_mode=PM.DoubleRow,
                        )
                nc.tensor.matmul(
                    moe_ps[m][:],
                    lhsT=m16t[:, m16_l["sv"][0] + m * P:
                              m16_l["sv"][0] + (m + 1) * P][0:SHF, :],
                    rhs=gs16[:SHF, :],
                    start=False, stop=True,
                )
                nc.vector.tensor_scalar(fin[:, m, :], moe_ps[m][:],
                                        float(rs_fsc), None, op0=Alu.mult)
            nc.sync.dma_start(
                rs_in[:].rearrange("(m p) s -> p m s", p=P), fin[:])

            # ---------- ReduceScatter of MoE + exact local residual ---
            rs_out = drp.tile((DSH, S), rs_dt, name="rs_out")
            nc.gpsimd.collective_compute(
                "ReduceScatter", Alu.add,
                replica_groups=[list(range(NCORES))],
                ins=[rs_in[:]], outs=[rs_out[:]],
            )
            rs_sb = cst.tile((DSH, S), rs_dt, name="rs_sb", tag="rs_sb")
            nc.sync.dma_start(rs_sb[:], rs_out[:])
            out32 = cst.tile((DSH, S), f32, name="out32", tag="out32")
            nc.vector.scalar_tensor_tensor(
                out32[:], rs_sb[:], float(1.0 / rs_isc), xsel_ps[:],
                op0=Alu.mult, op1=Alu.add,
            )
            nc.sync.dma_start(outT_d[:], out32[:])

    nc.compile()
    return nc


def _prep_inputs(inputs):
    """Host-side sharding/layout prep. Returns (in_maps, a1, a2)."""
    x = np.asarray(inputs["x"], np.float32)            # [1,S,D]
    attn_mask = np.asarray(inputs["attn_mask"])        # [1,S]
    wq = np.asarray(inputs["wq"], np.float32)
    wk = np.asarray(inputs["wk"], np.float32)
    wv = np.asarray(inputs["wv"], np.float32)
    wo = np.asarray(inputs["wo"], np.float32)
    a1 = float(np.asarray(inputs["a1"]).reshape(-1)[0])
    g1 = np.asarray(inputs["g1"], np.float32).reshape(Dm)
    b1 = np.asarray(inputs["b1"], np.float32).reshape(Dm)
    a2 = float(np.asarray(inputs["a2"]).reshape(-1)[0])
    g2 = np.asarray(inputs["g2"], np.float32).reshape(Dm)
    b2 = np.asarray(inputs["b2"], np.float32).reshape(Dm)
    gate_w = np.asarray(inputs["gate_w"], np.float32)  # [D,E]
    gate_b = np.asarray(inputs["gate_b"], np.float32).reshape(E)
    ek = np.asarray(inputs["ek"], np.float32)          # [E,D,FF]
    ev = np.asarray(inputs["ev"], np.float32)          # [E,FF,D]
    sk = np.asarray(inputs["sk"], np.float32)          # [1,D,FF]
    sv = np.asarray(inputs["sv"], np.float32)          # [1,FF,D]

    xT = np.ascontiguousarray(x[0].T)                  # [D,S]
    xTp = np.concatenate([xT[i * P:(i + 1) * P, :] for i in range(DC)], axis=1)

    # rope tables (transposed layout: [freq, pos]); the q/k halves carry
    # the fp8 weight descales (q: 1/(64*4), k: 1/(32*4))
    pos = np.arange(S, dtype=np.float32)
    half = HD // 2
    inv = 1.0 / (10000.0 ** (np.arange(half, dtype=np.float32) / half))
    ang = pos[:, None] * inv[None, :]                  # [S, half]
    cosT = np.cos(ang).T.astype(np.float32)            # [32,S]
    sinT = np.sin(ang).T.astype(np.float32)
    cd = np.concatenate([cosT / 256.0, cosT / 256.0,
                         cosT / 128.0, cosT / 128.0], 0)
    cs = np.concatenate([-sinT / 256.0, sinT / 256.0,
                         -sinT / 128.0, sinT / 128.0], 0)

    # additive attention mask, exactly as the reference builds it;
    # packed TRANSPOSED: block j = addmask.T[j*P:(j+1)*P, j*P:S]
    causal = np.tril(np.ones((S, S), np.float32))
    am = attn_mask.astype(np.float32)[0]               # [S]
    cm = causal * am[None, :]
    cm[np.arange(S), np.arange(S)] = 1.0
    cmT = cm.T                                         # [k, q] 0/1 visibility
    compact_mask = bool(np.all(am == 1.0))
    if compact_mask:
        mblocks = [cmT[0:P, 0:P]]
    else:
        mblocks = []
        for j in range(TCH):
            blk = np.zeros((P, S), np.float32)
            blk[:, : S - j * P] = cmT[j * P:(j + 1) * P, j * P:]
            mblocks.append(blk)

    (atn_l, atn_c), (at8_l, at8_c), (mid_l, mid_c), (mo8_l, mo8_c), \
        (m16_l, m16_c), (p32_l, p32_c) = _layouts(compact_mask)

    def pack(layout, total, blocks, dtype):
        arr = np.zeros((P, total), dtype)
        for name, data in blocks.items():
            off, cols = layout[name]
            data = np.asarray(data, np.float32)
            assert data.shape[1] == cols, (name, data.shape, cols)
            arr[:data.shape[0], off:off + cols] = data.astype(dtype)
        return arr

    def cat(chunks):
        return np.concatenate(chunks, axis=1)

    wo_pk = cat([wo[i * P:(i + 1) * P, :] * SWE for i in range(DC)])
    id128 = np.eye(P, dtype=np.float32)

    mid_pack = pack(mid_l, mid_c, {"wo": wo_pk}, F8)

    common32 = {
        "g1s": np.stack([g1[i * P:(i + 1) * P] * SH for i in range(DC)], 1),
        "b1s": np.stack([b1[i * P:(i + 1) * P] * SH for i in range(DC)], 1),
        "g2": np.stack([g2[i * P:(i + 1) * P] for i in range(DC)], 1),
        "b2": np.stack([b2[i * P:(i + 1) * P] for i in range(DC)], 1),
        "g2s": np.stack([g2[i * P:(i + 1) * P] * SH for i in range(DC)], 1),
        "b2s": np.stack([b2[i * P:(i + 1) * P] * SH for i in range(DC)], 1),
        "idf": id128,
    }

    in_maps = []
    for c in range(NCORES):
        perm = [2 * c, 2 * c + 1] + [e for e in range(E)
                                     if e not in (2 * c, 2 * c + 1)]
        gwp = gate_w[:, perm]
        gbp = gate_b[perm]
        hsl = slice(c * HD, (c + 1) * HD)
        wqk = np.concatenate([wq[:, hsl] * (0.125 * 64.0),
                              wk[:, hsl] * 32.0], axis=1)
        # rotate-half: same columns with each head's halves swapped
        swp = np.r_[32:64, 0:32, 96:128, 64:96]
        wqs = wqk[:, swp]
        wqk_pk = cat([wqk[i * P:(i + 1) * P, :] for i in range(DC)])
        wqs_pk = cat([wqs[i * P:(i + 1) * P, :] for i in range(DC)])
        wv_pk = cat([wv[i * P:(i + 1) * P, hsl] * SWE for i in range(DC)])
        at8_pack = pack(at8_l, at8_c, {
            "wqk": wqk_pk, "wqs": wqs_pk, "wv": wv_pk,
            "gw8": cat([gwp[i * P:(i + 1) * P, :] * SWE for i in range(DC)]),
        }, F8)
        atn_pack = pack(atn_l, atn_c, {
            "cd": cd, "cs": cs,
            "maskT": cat(mblocks), "idbf": id128,
            "ones": np.ones((P, P), np.float32),
            "c64": np.full((1, HD), 64.0, np.float32),
        }, BF)

        selm = np.zeros((P, DC * DSH), np.float32)
        for m in range(DC):
            for p in range(P):
                d = m * P + p
                if 64 * c <= d < 64 * c + DSH:
                    selm[p, m * DSH + (d - 64 * c)] = 1.0
        p32_pack = pack(p32_l, p32_c, dict(
            common32,
            gb=np.tile(gbp, (P, TCH)),
            selm=selm,
        ), np.float32)

        ek_pk = cat([ek[2 * c + e][i * P:(i + 1) * P, :] * SWE
                     for e in range(2) for i in range(DC)])
        ev_pk = cat([ev[2 * c + e][i * P:(i + 1) * P, :] * SV
                     for e in range(2) for i in range(FCH)])
        sk_pk = cat([sk[0][i * P:(i + 1) * P, c * SHF:(c + 1) * SHF] * SWE
                     for i in range(DC)])
        mo8_pack = pack(mo8_l, mo8_c, {
            "ek": ek_pk, "ev": ev_pk, "sk": sk_pk,
        }, F8)
        m16_pack = pack(m16_l, m16_c, {
            "sv": sv[0][c * SHF:(c + 1) * SHF, :] * SVSH,
        }, BF)

        in_maps.append(dict(
            xT=xTp.astype(np.float32),
            atn16=atn_pack, atn8=at8_pack, mid8=mid_pack, moe8=mo8_pack,
            moe16=m16_pack, p32=p32_pack,
        ))
    return in_maps, a1, a2, compact_mask


def kernel(**inputs):
    from concourse import bass_utils

    sim = bool(os.environ.get("BASSK_SIM"))
    sim_gelu = sim or bool(os.environ.get("BASSK_COMPOSED_GELU"))
    rs8 = not bool(os.environ.get("BASSK_NO_RS8"))
    in_maps, a1v, a2v, compact_mask = _prep_inputs(inputs)
    key = (a1v, a2v, sim_gelu, compact_mask, rs8)
    if key not in _PROG_CACHE:
        _PROG_CACHE[key] = _build_program(a1v, a2v, sim_gelu,
                                          compact_mask, rs8)
    nc = _PROG_CACHE[key]

    if sim:
        from concourse.bass_interp import MultiCoreSim

        simu = MultiCoreSim(nc, num_cores=NCORES)
        for c in range(NCORES):
            for k, v in in_maps[c].items():
                simu.cores[c].tensor(k)[:] = v
        simu.simulate(check_with_hw=False)
        shards = [np.array(simu.cores[c].tensor("outT")) for c in range(NCORES)]
    else:
        trace = bool(os.environ.get("BASSK_TRACE"))
        # Optional warm-up execution (measured: does not reduce the
        # inter-core dispatch stagger in this environment; off by default)
        if os.environ.get("BASSK_WARMRUN"):
            bass_utils.run_bass_kernel_spmd(
                nc, in_maps, core_ids=list(range(NCORES)), trace=False
            )
        res = bass_utils.run_bass_kernel_spmd(
            nc, in_maps, core_ids=list(range(NCORES)), trace=trace
        )
        LAST_INFO["exec_time_ns"] = res.exec_time_ns
        LAST_INFO["profile_json"] = res.profile_json
        shards = [np.asarray(res.results[c]["outT"]) for c in range(NCORES)]

    outT = np.concatenate(shards, axis=0)              # [D,S]
    return np.ascontiguousarray(outT.T).reshape(1, S, Dm).astype(np.float32)


# revision 4
# speedup vs baseline: 1.0815x; 1.0815x over previous
"""Trainium2 Bass kernel v2: DyT-prenorm attention (RoPE, causal+mask) +
top-2-of-16 MoE with a shared expert.

Differences vs v1 baseline:
  * Attention computes scores TRANSPOSED (scT[k,q] = krot^T qrot), so the
    exp output feeds the P@V matmul directly as the moving operand -- no
    per-block PE transposes / DVE copies.  Softmax skips the max-subtract
    (scores are O(1) for this problem) and the denominator comes free from
    a 65th all-ones column appended to V.  Normalization is applied once at
    the end via a rank-1 (ones x 1/sum) PE broadcast.
  * Experts (ek/ev), shared-expert up (sk) and their activations run in
    fp8e4m3 with DoubleRow matmuls (2 contraction chunks per MM).  Scales
    are folded host-side: ek*32, sk*32, h2*4 (via g2/b2*4), gelu scale
    1/128, gates*16, ev*16, shared sv(bf16)*256, final copy *1/256.
  * A tiny barrier AllGather is issued at t=0 (overlaps the weight loads)
    to absorb inter-core launch skew / first-collective setup cost.
  * Dummy PE matmuls keep the tensor engine HAM-warm through the mid-kernel
    AllGather stall; tiny dummy activations prefetch the ACT function
    tables (tanh/exp/gelu) before the real uses.
"""

import os
import numpy as np
import ml_dtypes

BF = ml_dtypes.bfloat16
F8 = ml_dtypes.float8_e4m3

S = 512      # tokens (B=1)
Dm = 512     # d_model
H = 8        # heads
HD = 64      # head dim
E = 16       # experts
FF = 512     # expert hidden
P = 128
NCORES = 8
DC = Dm // P    # 4 d-model chunks
TCH = S // P    # 4 token chunks
FCH = FF // P   # 4 ff chunks
SHF = FF // NCORES  # shared-expert ff slice per core (64)
DSH = Dm // NCORES  # output row shard per core (64)

# fp8 scale folding
SH = 4.0      # h2 fp8 scale (folded into g2/b2)
SWE = 32.0    # ek/sk fp8 scale
SV = 16.0     # ev fp8 scale
SG = 16.0     # gate scale (folded into rep16)
GELU_SC = 1.0 / (SH * SWE)    # up-projection descale before gelu
FIN_SC = 1.0 / (SG * SV)      # final moe descale
SVSH = SG * SV                # shared-expert sv (bf16) pre-scale

GELU_C = float(2.0 * np.sqrt(2.0 / np.pi))
GELU_A = 0.044715

_PROG_CACHE = {}
LAST_INFO = {}


def _layouts(compact_mask):
    """Column layouts of the packed constant arrays (shared host/device)."""
    def lay(blocks):
        off, out = 0, {}
        for name, cols in blocks:
            out[name] = (off, cols)
            off += cols
        return out, off

    atn, atn_c = lay([
        ("cd", S), ("cs", S),
        ("maskT", P if compact_mask else TCH * S), ("idbf", P), ("ones", P),
        ("c64", HD),
    ])
    at8, at8_c = lay([
        ("wqk", DC * P), ("wqs", DC * P), ("wv", DC * HD), ("gw8", DC * E),
    ])
    mid, mid_c = lay([("wo", DC * Dm)])
    mo8, mo8_c = lay([
        ("ek", 2 * DC * FF), ("ev", 2 * FCH * Dm), ("sk", DC * SHF),
    ])
    m16, m16_c = lay([("sv", Dm)])
    p32, p32_c = lay([
        ("g1s", DC), ("b1s", DC), ("g2", DC), ("b2", DC),
        ("g2s", DC), ("b2s", DC),
        ("gb", TCH * E), ("smsk", 2 * DC),
        ("idf", P),
    ])
    return (atn, atn_c), (at8, at8_c), (mid, mid_c), (mo8, mo8_c), \
        (m16, m16_c), (p32, p32_c)


def _build_program(a1v: float, a2v: float, sim_gelu: bool,
                   compact_mask: bool, rs8: bool):
    import concourse.bass as bass
    import concourse.mybir as mybir
    import concourse.tile as tile
    from concourse import bacc

    f32 = mybir.dt.float32
    bf16 = mybir.dt.bfloat16
    fp8 = mybir.dt.float8e4
    Alu = mybir.AluOpType
    Act = mybir.ActivationFunctionType
    PM = mybir.MatmulPerfMode
    AX = mybir.AxisListType
    ts = bass.ts

    (atn_l, atn_c), (at8_l, at8_c), (mid_l, mid_c), (mo8_l, mo8_c), \
        (m16_l, m16_c), (p32_l, p32_c) = _layouts(compact_mask)

    nc = bacc.Bacc(
        "TRN2", target_bir_lowering=False, debug=False, num_devices=NCORES
    )

    def inp(name, shape, dt=f32):
        return nc.dram_tensor(name, list(shape), dt, kind="ExternalInput").ap()

    xT_d = inp("xT", (P, DC * S))
    atn_d = inp("atn16", (P, atn_c), bf16)
    at8_d = inp("atn8", (P, at8_c), fp8)
    mid_d = inp("mid8", (P, mid_c), fp8)
    mo8_d = inp("moe8", (P, mo8_c), fp8)
    m16_d = inp("moe16", (P, m16_c), bf16)
    p32_d = inp("p32", (P, p32_c))

    outT_d = nc.dram_tensor("outT", [DSH, S], f32, kind="ExternalOutput").ap()

    with tile.TileContext(nc, num_cores=NCORES) as tc:
        with (
            tc.tile_pool(name="cst", bufs=1) as cst,
            tc.tile_pool(name="tmp", bufs=3) as tmp,
            tc.tile_pool(name="ps", bufs=2, space="PSUM") as psp,
            tc.tile_pool(name="dram", bufs=1, space="DRAM") as drp,
        ):
            # ---------- packed loads (7 DMA dispatches, 2 queues) ----------
            xTt = cst.tile((P, DC * S), f32, name="xTt", tag="xTt")
            nc.sync.dma_start(xTt[:, 0:2 * S], xT_d[:, 0:2 * S])
            pk32 = cst.tile((P, p32_c), f32, name="pk32", tag="pk32")
            nc.scalar.dma_start(pk32[:], p32_d[:])
            at8 = cst.tile((P, at8_c), fp8, name="at8", tag="at8")
            nc.sync.dma_start(at8[:], at8_d[:])
            atn16 = cst.tile((P, atn_c), bf16, name="atn16", tag="atn16")
            nc.sync.dma_start(atn16[:], atn_d[:])
            nc.sync.dma_start(xTt[:, 2 * S:4 * S], xT_d[:, 2 * S:4 * S])
            mid8 = cst.tile((P, mid_c), fp8, name="mid8", tag="mid8")
            nc.scalar.dma_start(mid8[:], mid_d[:])
            moe8 = cst.tile((P, mo8_c), fp8, name="moe8", tag="moe8")
            nc.sync.dma_start(moe8[:], mo8_d[:])
            m16t = cst.tile((P, m16_c), bf16, name="m16t", tag="m16t")
            nc.scalar.dma_start(m16t[:], m16_d[:])

            def asl(name, c=0, w=None):  # attention-pack slice
                off, cols = atn_l[name]
                w = cols if w is None else w
                return atn16[:, off + c * w: off + (c + 1) * w]

            def a8l(name, c=0, w=None):  # fp8 attention-pack slice
                off, cols = at8_l[name]
                w = cols if w is None else w
                return at8[:, off + c * w: off + (c + 1) * w]

            def psl(name, c=0, w=None):  # fp32-pack slice
                off, cols = p32_l[name]
                w = cols if w is None else w
                return pk32[:, off + c * w: off + (c + 1) * w]

            idbf = asl("idbf")
            ones16 = asl("ones")
            idf = psl("idf")
            # 3D chunk-major views of the fp8 weight packs
            ek8 = moe8[:, mo8_l["ek"][0]: mo8_l["ek"][0] + 2 * DC * FF]
            ek8 = ek8.rearrange("p (c f) -> p c f", c=2 * DC)
            ev8 = moe8[:, mo8_l["ev"][0]: mo8_l["ev"][0] + 2 * FCH * Dm]
            ev8 = ev8.rearrange("p (c f) -> p c f", c=2 * FCH)
            sk8 = moe8[:, mo8_l["sk"][0]: mo8_l["sk"][0] + DC * SHF]
            sk8 = sk8.rearrange("p (c f) -> p c f", c=DC)
            wqk8 = a8l("wqk").rearrange("p (c f) -> p c f", c=DC)
            wqs8 = a8l("wqs").rearrange("p (c f) -> p c f", c=DC)
            wv8 = a8l("wv").rearrange("p (c f) -> p c f", c=DC)
            wo8 = mid8[:, mid_l["wo"][0]: mid_l["wo"][0] + DC * Dm]
            wo8 = wo8.rearrange("p (c f) -> p c f", c=DC)

            dum = tmp.tile((1, 8), f32, name="dum", tag="dum", bufs=1)

            # ---------- phase 1: dyt1 + QK/V ----------
            hT8 = cst.tile((P, DC, S), fp8, name="hT8", tag="hT8")
            for c in range(DC):
                th = tmp.tile((P, S), f32, name="th", tag="t32")
                nc.scalar.activation(th[:], xTt[:, ts(c, S)], Act.Tanh,
                                     scale=float(a1v))
                nc.vector.scalar_tensor_tensor(
                    hT8[:, c, :], th[:], psl("g1s", c, 1),
                    psl("b1s", c, 1).to_broadcast((P, S)),
                    op0=Alu.mult, op1=Alu.add,
                )
            # prefetch the Exp table while QK/V matmuls run
            nc.scalar.activation(dum[:], pk32[0:1, 0:8], Act.Exp)

            # qkT = [wq*0.125 | wk]^T @ h  -> [128 (q64|k64), S], plus the
            # same with per-head halves pre-swapped (for rope's rotate-half)
            qk_ps = psp.tile((P, S), f32, name="qk_ps", tag="mm")
            for j in range(2):
                nc.tensor.matmul(
                    qk_ps[:], lhsT=wqk8[:, 2 * j:2 * j + 2, :],
                    rhs=hT8[:, 2 * j:2 * j + 2, :],
                    start=(j == 0), stop=(j == 1), perf_mode=PM.DoubleRow,
                )
            qs_ps = psp.tile((P, S), f32, name="qs_ps", tag="mm")
            for j in range(2):
                nc.tensor.matmul(
                    qs_ps[:], lhsT=wqs8[:, 2 * j:2 * j + 2, :],
                    rhs=hT8[:, 2 * j:2 * j + 2, :],
                    start=(j == 0), stop=(j == 1), perf_mode=PM.DoubleRow,
                )

            # v (token-major) in one PSUM tile [128, 4, 64]; SBUF copy gets
            # a 65th all-ones column (the softmax-denominator trick).
            v_ps = psp.tile((P, TCH, HD), f32, name="v_ps", tag="avT",
                            bufs=1)
            for t in range(TCH):
                for j in range(2):
                    nc.tensor.matmul(
                        v_ps[:, t, :], lhsT=hT8[:, 2 * j:2 * j + 2, ts(t, P)],
                        rhs=wv8[:, 2 * j:2 * j + 2, :],
                        start=(j == 0), stop=(j == 1), perf_mode=PM.DoubleRow,
                    )
            v16 = []
            for t in range(TCH):
                vt = cst.tile((P, HD + 1), bf16, name=f"v16_{t}",
                              tag=f"v16_{t}")
                nc.any.tensor_scalar(vt[:, 0:HD], v_ps[:, t, :],
                                     1.0 / 128.0, None, op0=Alu.mult)
                nc.vector.memset(vt[:, HD:HD + 1], 1.0)
                v16.append(vt)

            # rope: qkrot = qk*cos + swapped_qk*(+-sin), 3 full-width DVE ops
            r1 = tmp.tile((P, S), f32, name="r1", tag="t32")
            nc.vector.tensor_tensor(r1[:], qk_ps[:], asl("cd"), Alu.mult)
            sw = tmp.tile((P, S), f32, name="sw", tag="t32")
            nc.vector.tensor_tensor(sw[:], qs_ps[:], asl("cs"), Alu.mult)
            qrot_t = cst.tile((HD, S), bf16, name="qrot", tag="qrot")
            nc.vector.tensor_tensor(qrot_t[:], r1[0:HD, :], sw[0:HD, :],
                                    Alu.add)
            krot_t = cst.tile((HD, S), bf16, name="krot", tag="krot")
            nc.vector.tensor_tensor(krot_t[:], r1[HD:P, :], sw[HD:P, :],
                                    Alu.add)
            qrot = qrot_t[:]
            krot = krot_t[:]

            # ---------- transposed scores + unnormalized softmax + A@V ----
            # avT_ps rows 0:64 = V^T @ exp(S^T); row 64 = colsums (denom).
            avT_ps = psp.tile((HD + 1, S), f32, name="avT_ps", tag="avT",
                              bufs=1)
            uT16 = []
            for j in range(TCH):
                L = S - j * P
                scT_ps = psp.tile((P, S), f32, name="scT_ps", tag="mm")
                nc.tensor.matmul(
                    scT_ps[:, :L], lhsT=krot[:, ts(j, P)],
                    rhs=qrot[:, j * P:S], start=True, stop=True,
                )
                u = tmp.tile((P, S), bf16, name=f"uT_{j}", tag=f"uT_{j}",
                             bufs=1)
                nc.scalar.activation(u[:, :L], scT_ps[:, :L], Act.Exp)
                # masking as a 0/1 multiply on the exp output (GPSIMD can't
                # touch PSUM); exp(s)*m == exp(s + log m) for m in {0,1}
                if compact_mask:
                    # only the diagonal block needs masking (identical for
                    # every j); off-diag causal blocks are fully visible
                    nc.any.tensor_tensor(
                        u[:, :P], u[:, :P], asl("maskT"), Alu.mult,
                    )
                else:
                    nc.any.tensor_tensor(
                        u[:, :L], u[:, :L],
                        asl("maskT", j, S)[:, :L], Alu.mult,
                    )
                uT16.append(u)
                # query-region j is complete once exp_j exists: accumulate
                # its k-chunks jj=0..j as one contiguous PSUM group
                for jj in range(j + 1):
                    nc.tensor.matmul(
                        avT_ps[:, ts(j, P)], lhsT=v16[jj][:],
                        rhs=uT16[jj][:, (j - jj) * P:(j - jj + 1) * P],
                        start=(jj == 0), stop=(jj == j),
                    )
            # prefetch the Tanh table (dyt2) while the AllGather runs
            nc.scalar.activation(dum[:], pk32[0:1, 0:8], Act.Tanh)

            # normalization: rinv = 1/colsum; rank-1 broadcast over 64 rows
            rinv16 = cst.tile((1, S), bf16, name="rinv16", tag="rinv16")
            with nc.allow_low_precision(reason="softmax denom, bf16 is fine"):
                nc.vector.reciprocal(rinv16[:], avT_ps[HD:HD + 1, :])
            # rep = 64/sum (the 64 pre-scales ao for fp8; descaled after wo)
            rep_ps = psp.tile((HD, S), f32, name="rep_ps", tag="lg", bufs=1)
            nc.tensor.matmul(rep_ps[:], lhsT=asl("c64")[0:1, :],
                             rhs=rinv16[:], start=True, stop=True)
            rep_s = tmp.tile((HD, S), bf16, name="rep_s", tag="rep_s",
                             bufs=1)
            nc.vector.tensor_copy(rep_s[:], rep_ps[:])
            ao16 = cst.tile((HD, S), fp8, name="ao16", tag="ao16")
            nc.vector.tensor_tensor(ao16[:], avT_ps[0:HD, :], rep_s[:],
                                    Alu.mult)

            # ---------- AllGather attention outputs (heads, fp8) ----------
            ag_in = drp.tile((HD, S), fp8, name="ag_in")
            ag_out = drp.tile((H * HD, S), fp8, name="ag_out",
                              addr_space="Shared")
            nc.sync.dma_start(ag_in[:], ao16[:])
            nc.gpsimd.collective_compute(
                "AllGather", Alu.bypass,
                replica_groups=[list(range(NCORES))],
                ins=[ag_in[:]], outs=[ag_out[:]],
            )
            aoT = cst.tile((P, DC, S), fp8, name="aoT", tag="aoT")
            nc.sync.dma_start(
                aoT[:], ag_out[:].rearrange("(c p) s -> p c s", p=P))

            # ---------- wo projection + residual + dyt2 ----------
            h28 = cst.tile((P, DC, S), fp8, name="h28", tag="h28")
            x1T32 = []
            for m in range(DC):
                pw = psp.tile((P, S), f32, name="pw", tag="mm")
                for j in range(2):
                    nc.tensor.matmul(
                        pw[:], lhsT=wo8[:, 2 * j:2 * j + 2, m * P:(m + 1) * P],
                        rhs=aoT[:, 2 * j:2 * j + 2, :],
                        start=(j == 0), stop=(j == 1),
                        perf_mode=PM.DoubleRow,
                    )
                x1 = cst.tile((P, S), f32, name=f"x1T{m}", tag=f"x1T{m}")
                nc.vector.scalar_tensor_tensor(
                    x1[:], pw[:], 1.0 / 2048.0, xTt[:, ts(m, S)],
                    op0=Alu.mult, op1=Alu.add,
                )
                x1T32.append(x1)
                th = tmp.tile((P, S), f32, name="th2", tag="t32")
                nc.scalar.activation(th[:], x1[:], Act.Tanh, scale=float(a2v))
                nc.vector.scalar_tensor_tensor(
                    h28[:, m, :], th[:], psl("g2s", m, 1),
                    psl("b2s", m, 1).to_broadcast((P, S)),
                    op0=Alu.mult, op1=Alu.add,
                )
            # prefetch the Exp table (router softmax)
            nc.scalar.activation(dum[:], pk32[0:1, 0:8], Act.Exp)

            # selected x1 rows [64c, 64c+64) (one-hot smsk), added after RS
            xsel = cst.tile((DSH, S), f32, name="xsel", tag="xsel")
            nc.any.tensor_scalar(
                xsel[:], x1T32[0][0:DSH, :], psl("smsk", 0, 1)[0:DSH, :],
                None, op0=Alu.mult,
            )
            for jj in range(1, 2 * DC):
                m, hh = jj // 2, jj % 2
                if hh == 0:
                    src_ap = x1T32[m][0:DSH, :]
                else:
                    x1h = cst.tile((DSH, S), f32, name=f"x1h{m}",
                                   tag=f"x1h{m}")
                    nc.any.tensor_copy(x1h[:], x1T32[m][DSH:P, :])
                    src_ap = x1h[:]
                nc.vector.scalar_tensor_tensor(
                    xsel[:], src_ap, psl("smsk", jj, 1)[0:DSH, :], xsel[:],
                    op0=Alu.mult, op1=Alu.add,
                )

            # ---------- router (fp32) ----------
            lg_ps = psp.tile((P, TCH, E), f32, name="lg_ps", tag="lg", bufs=1)
            gw8v = a8l("gw8").rearrange("p (c e) -> p c e", c=DC)
            for t in range(TCH):
                for j in range(2):
                    nc.tensor.matmul(
                        lg_ps[:, t, :], lhsT=h28[:, 2 * j:2 * j + 2, ts(t, P)],
                        rhs=gw8v[:, 2 * j:2 * j + 2, :],
                        start=(j == 0), stop=(j == 1),
                        perf_mode=PM.DoubleRow,
                    )

            # ---------- expert up-projections (fp8 DoubleRow) ----------
            def gelu_scaled(dst_rows, src_ps, rows):
                """gelu_tanh(GELU_SC * src_ps[:rows]) -> bf16 tile."""
                g0 = tmp.tile((P, S), bf16, name="g0", tag="g16", bufs=10)
                if not sim_gelu:
                    nc.scalar.activation(g0[:rows], src_ps[:rows],
                                         Act.Gelu_apprx_tanh, scale=GELU_SC)
                    return g0
                u16 = tmp.tile((P, S), bf16, name="u16", tag="u16", bufs=3)
                nc.vector.tensor_scalar(u16[:rows], src_ps[:rows], GELU_SC,
                                        None, op0=Alu.mult)
                x2 = tmp.tile((P, S), bf16, name="x2", tag="x2", bufs=3)
                nc.vector.tensor_tensor(x2[:rows], u16[:rows], u16[:rows],
                                        Alu.mult)
                t1 = tmp.tile((P, S), bf16, name="t1", tag="x2", bufs=3)
                nc.vector.tensor_scalar(
                    t1[:rows], x2[:rows], GELU_A, 1.0,
                    op0=Alu.mult, op1=Alu.add,
                )
                mm_ = tmp.tile((P, S), bf16, name="mm_", tag="x2", bufs=3)
                nc.vector.tensor_tensor(mm_[:rows], u16[:rows], t1[:rows],
                                        Alu.mult)
                sg = tmp.tile((P, S), bf16, name="sg", tag="x2", bufs=3)
                nc.scalar.activation(sg[:rows], mm_[:rows], Act.Sigmoid,
                                     scale=GELU_C)
                nc.vector.tensor_tensor(g0[:rows], u16[:rows], sg[:rows],
                                        Alu.mult)
                return g0

            g0s = [[], []]
            for el in range(2):
                for fc in range(FCH):
                    up_ps = psp.tile((P, S), f32, name="up_ps", tag="mm")
                    for j in range(2):
                        nc.tensor.matmul(
                            up_ps[:],
                            lhsT=ek8[:, el * DC + 2 * j: el * DC + 2 * j + 2,
                                     fc * P:(fc + 1) * P],
                            rhs=h28[:, 2 * j:2 * j + 2, :],
                            start=(j == 0), stop=(j == 1),
                            perf_mode=PM.DoubleRow,
                        )
                    g0s[el].append(gelu_scaled(None, up_ps, P))
            # shared-expert up (fp8 DoubleRow)
            su_ps2 = psp.tile((SHF, S), f32, name="su_ps2", tag="mm")
            for j in range(2):
                nc.tensor.matmul(
                    su_ps2[:], lhsT=sk8[:, 2 * j:2 * j + 2, :],
                    rhs=h28[:, 2 * j:2 * j + 2, :],
                    start=(j == 0), stop=(j == 1),
                    perf_mode=PM.DoubleRow,
                )
            gs16 = gelu_scaled(None, su_ps2, SHF)

            # ---------- top-2 gates (DVE, overlaps the ups) ----------
            gb_ap = psl("gb").rearrange("p (t e) -> p t e", e=E)
            lg32 = cst.tile((P, TCH, E), f32, name="lg32", tag="lg32")
            nc.vector.scalar_tensor_tensor(
                lg32[:], lg_ps[:], 1.0 / 128.0, gb_ap,
                op0=Alu.mult, op1=Alu.add,
            )
            ex32 = cst.tile((P, TCH, E), f32, name="ex32", tag="ex32")
            nc.scalar.activation(ex32[:], lg32[:], Act.Exp, scale=1.0)
            ssum4 = cst.tile((P, TCH), f32, name="ssum4", tag="ssum4")
            nc.vector.reduce_sum(ssum4[:], ex32[:], axis=AX.X)
            rinv4 = cst.tile((P, TCH), f32, name="rinv4", tag="rinv4")
            nc.vector.reciprocal(rinv4[:], ssum4[:])
            # prefetch the Gelu table while the top-2 chain runs
            if not sim_gelu:
                nc.scalar.activation(dum[:], pk32[0:1, 0:8],
                                     Act.Gelu_apprx_tanh)
            prb = cst.tile((P, TCH, E), f32, name="prb", tag="prb")
            nc.vector.tensor_tensor(
                prb[:], ex32[:], rinv4[:, :, None].to_broadcast((P, TCH, E)),
                Alu.mult,
            )
            m1 = cst.tile((P, TCH), f32, name="m1", tag="m1")
            nc.vector.reduce_max(m1[:], prb[:], axis=AX.X)
            ge1 = cst.tile((P, TCH, E), f32, name="ge1", tag="ge1")
            nc.vector.tensor_tensor(
                ge1[:], prb[:], m1[:, :, None].to_broadcast((P, TCH, E)),
                Alu.is_ge,
            )
            msk = cst.tile((P, TCH, E), f32, name="msk", tag="msk")
            nc.vector.scalar_tensor_tensor(
                msk[:], ge1[:], -1e9, prb[:], op0=Alu.mult, op1=Alu.add
            )
            m2 = cst.tile((P, TCH), f32, name="m2", tag="m2")
            nc.vector.reduce_max(m2[:], msk[:], axis=AX.X)
            ge2 = cst.tile((P, TCH, E), f32, name="ge2", tag="ge2")
            nc.vector.tensor_tensor(
                ge2[:], prb[:], m2[:, :, None].to_broadcast((P, TCH, E)),
                Alu.is_ge,
            )
            wg = cst.tile((P, TCH, E), f32, name="wg", tag="wg")
            nc.vector.tensor_tensor(wg[:], prb[:], ge2[:], Alu.mult)

            # transpose the two local experts' gate columns to rows
            wrow = [
                cst.tile((1, S), bf16, name=f"wrow{el}", tag=f"wrow{el}")
                for el in range(2)
            ]
            for t in range(TCH):
                for el in range(2):
                    wt_ps = psp.tile((1, P), f32, name="wt_ps",
                                     tag="lg", bufs=1)
                    nc.tensor.transpose(wt_ps[:], wg[:, t, el:el + 1], idf)
                    nc.vector.tensor_copy(wrow[el][:, ts(t, P)], wt_ps[:])

            # broadcast local-expert gate rows across partitions (rank-1 mm),
            # scaled by SG for the fp8 down-projection.
            rep16 = []
            for el in range(2):
                rp_ps = psp.tile((P, S), f32, name="rp_ps", tag="mm")
                nc.tensor.matmul(
                    rp_ps[:], lhsT=ones16[0:1, :], rhs=wrow[el][:],
                    start=True, stop=True,
                )
                rp = cst.tile((P, S), bf16, name=f"rep16_{el}",
                              tag=f"rep16_{el}")
                nc.vector.tensor_scalar(rp[:], rp_ps[:], float(SG), None,
                                        op0=Alu.mult)
                rep16.append(rp)

            # ---------- gate the gelu outputs (fp8), down-projections -----
            g8 = cst.tile((P, 2 * FCH, S), fp8, name="g8", tag="g8")
            for el in range(2):
                for fc in range(FCH):
                    nc.gpsimd.tensor_tensor(
                        g8[:, el * FCH + fc, :], g0s[el][fc][:], rep16[el][:],
                        Alu.mult,
                    )
            moe_ps = [
                psp.tile((P, S), f32, name=f"moe_ps{m}", tag="moe", bufs=4)
                for m in range(DC)
            ]
            # fp8 RS halves the collective bytes; payload pre-scaled by
            # rs_isc so fp8 keeps ~2 significant digits of the partials
            rs_dt = fp8 if rs8 else bf16
            rs_isc = 16.0 if rs8 else 1.0
            rs_fsc = FIN_SC * rs_isc
            rs_in = drp.tile((Dm, S), rs_dt, name="rs_in")
            fin = tmp.tile((P, DC, S), rs_dt, name="fin", tag="fin", bufs=1)
            for m in range(DC):
                for el in range(2):
                    for j in range(2):
                        nc.tensor.matmul(
                            moe_ps[m][:],
                            lhsT=ev8[:, el * FCH + 2 * j: el * FCH + 2 * j + 2,
                                     m * P:(m + 1) * P],
                            rhs=g8[:, el * FCH + 2 * j: el * FCH + 2 * j + 2, :],
                            start=(el == 0 and j == 0), stop=False,
                            perf_mode=PM.DoubleRow,
                        )
                nc.tensor.matmul(
                    moe_ps[m][:],
                    lhsT=m16t[:, m16_l["sv"][0] + m * P:
                              m16_l["sv"][0] + (m + 1) * P][0:SHF, :],
                    rhs=gs16[:SHF, :],
                    start=False, stop=True,
                )
                nc.vector.tensor_scalar(fin[:, m, :], moe_ps[m][:],
                                        float(rs_fsc), None, op0=Alu.mult)
            nc.sync.dma_start(
                rs_in[:].rearrange("(m p) s -> p m s", p=P), fin[:])

            # ---------- ReduceScatter of MoE + exact local residual ---
            rs_out = drp.tile((DSH, S), rs_dt, name="rs_out")
            nc.gpsimd.collective_compute(
                "ReduceScatter", Alu.add,
                replica_groups=[list(range(NCORES))],
                ins=[rs_in[:]], outs=[rs_out[:]],
            )
            rs_sb = cst.tile((DSH, S), rs_dt, name="rs_sb", tag="rs_sb")
            nc.sync.dma_start(rs_sb[:], rs_out[:])
            out32 = cst.tile((DSH, S), f32, name="out32", tag="out32")
            nc.vector.scalar_tensor_tensor(
                out32[:], rs_sb[:], float(1.0 / rs_isc), xsel[:],
                op0=Alu.mult, op1=Alu.add,
            )
            nc.sync.dma_start(outT_d[:], out32[:])

    nc.compile()
    return nc


def _prep_inputs(inputs):
    """Host-side sharding/layout prep. Returns (in_maps, a1, a2)."""
    x = np.asarray(inputs["x"], np.float32)            # [1,S,D]
    attn_mask = np.asarray(inputs["attn_mask"])        # [1,S]
    wq = np.asarray(inputs["wq"], np.float32)
    wk = np.asarray(inputs["wk"], np.float32)
    wv = np.asarray(inputs["wv"], np.float32)
    wo = np.asarray(inputs["wo"], np.float32)
    a1 = float(np.asarray(inputs["a1"]).reshape(-1)[0])
    g1 = np.asarray(inputs["g1"], np.float32).reshape(Dm)
    b1 = np.asarray(inputs["b1"], np.float32).reshape(Dm)
    a2 = float(np.asarray(inputs["a2"]).reshape(-1)[0])
    g2 = np.asarray(inputs["g2"], np.float32).reshape(Dm)
    b2 = np.asarray(inputs["b2"], np.float32).reshape(Dm)
    gate_w = np.asarray(inputs["gate_w"], np.float32)  # [D,E]
    gate_b = np.asarray(inputs["gate_b"], np.float32).reshape(E)
    ek = np.asarray(inputs["ek"], np.float32)          # [E,D,FF]
    ev = np.asarray(inputs["ev"], np.float32)          # [E,FF,D]
    sk = np.asarray(inputs["sk"], np.float32)          # [1,D,FF]
    sv = np.asarray(inputs["sv"], np.float32)          # [1,FF,D]

    xT = np.ascontiguousarray(x[0].T)                  # [D,S]
    xTp = np.concatenate([xT[i * P:(i + 1) * P, :] for i in range(DC)], axis=1)

    # rope tables (transposed layout: [freq, pos]); the q/k halves carry
    # the fp8 weight descales (q: 1/(64*4), k: 1/(32*4))
    pos = np.arange(S, dtype=np.float32)
    half = HD // 2
    inv = 1.0 / (10000.0 ** (np.arange(half, dtype=np.float32) / half))
    ang = pos[:, None] * inv[None, :]                  # [S, half]
    cosT = np.cos(ang).T.astype(np.float32)            # [32,S]
    sinT = np.sin(ang).T.astype(np.float32)
    cd = np.concatenate([cosT / 256.0, cosT / 256.0,
                         cosT / 128.0, cosT / 128.0], 0)
    cs = np.concatenate([-sinT / 256.0, sinT / 256.0,
                         -sinT / 128.0, sinT / 128.0], 0)

    # additive attention mask, exactly as the reference builds it;
    # packed TRANSPOSED: block j = addmask.T[j*P:(j+1)*P, j*P:S]
    causal = np.tril(np.ones((S, S), np.float32))
    am = attn_mask.astype(np.float32)[0]               # [S]
    cm = causal * am[None, :]
    cm[np.arange(S), np.arange(S)] = 1.0
    cmT = cm.T                                         # [k, q] 0/1 visibility
    compact_mask = bool(np.all(am == 1.0))
    if compact_mask:
        mblocks = [cmT[0:P, 0:P]]
    else:
        mblocks = []
        for j in range(TCH):
            blk = np.zeros((P, S), np.float32)
            blk[:, : S - j * P] = cmT[j * P:(j + 1) * P, j * P:]
            mblocks.append(blk)

    (atn_l, atn_c), (at8_l, at8_c), (mid_l, mid_c), (mo8_l, mo8_c), \
        (m16_l, m16_c), (p32_l, p32_c) = _layouts(compact_mask)

    def pack(layout, total, blocks, dtype):
        arr = np.zeros((P, total), dtype)
        for name, data in blocks.items():
            off, cols = layout[name]
            data = np.asarray(data, np.float32)
            assert data.shape[1] == cols, (name, data.shape, cols)
            arr[:data.shape[0], off:off + cols] = data.astype(dtype)
        return arr

    def cat(chunks):
        return np.concatenate(chunks, axis=1)

    wo_pk = cat([wo[i * P:(i + 1) * P, :] * SWE for i in range(DC)])
    id128 = np.eye(P, dtype=np.float32)

    mid_pack = pack(mid_l, mid_c, {"wo": wo_pk}, F8)

    common32 = {
        "g1s": np.stack([g1[i * P:(i + 1) * P] * SH for i in range(DC)], 1),
        "b1s": np.stack([b1[i * P:(i + 1) * P] * SH for i in range(DC)], 1),
        "g2": np.stack([g2[i * P:(i + 1) * P] for i in range(DC)], 1),
        "b2": np.stack([b2[i * P:(i + 1) * P] for i in range(DC)], 1),
        "g2s": np.stack([g2[i * P:(i + 1) * P] * SH for i in range(DC)], 1),
        "b2s": np.stack([b2[i * P:(i + 1) * P] * SH for i in range(DC)], 1),
        "idf": id128,
    }

    in_maps = []
    for c in range(NCORES):
        perm = [2 * c, 2 * c + 1] + [e for e in range(E)
                                     if e not in (2 * c, 2 * c + 1)]
        gwp = gate_w[:, perm]
        gbp = gate_b[perm]
        hsl = slice(c * HD, (c + 1) * HD)
        wqk = np.concatenate([wq[:, hsl] * (0.125 * 64.0),
                              wk[:, hsl] * 32.0], axis=1)
        # rotate-half: same columns with each head's halves swapped
        swp = np.r_[32:64, 0:32, 96:128, 64:96]
        wqs = wqk[:, swp]
        wqk_pk = cat([wqk[i * P:(i + 1) * P, :] for i in range(DC)])
        wqs_pk = cat([wqs[i * P:(i + 1) * P, :] for i in range(DC)])
        wv_pk = cat([wv[i * P:(i + 1) * P, hsl] * SWE for i in range(DC)])
        at8_pack = pack(at8_l, at8_c, {
            "wqk": wqk_pk, "wqs": wqs_pk, "wv": wv_pk,
            "gw8": cat([gwp[i * P:(i + 1) * P, :] * SWE for i in range(DC)]),
        }, F8)
        atn_pack = pack(atn_l, atn_c, {
            "cd": cd, "cs": cs,
            "maskT": cat(mblocks), "idbf": id128,
            "ones": np.ones((P, P), np.float32),
            "c64": np.full((1, HD), 64.0, np.float32),
        }, BF)

        p32_pack = pack(p32_l, p32_c, dict(
            common32,
            gb=np.tile(gbp, (P, TCH)),
            smsk=np.tile((np.arange(2 * DC) == c).astype(np.float32), (P, 1)),
        ), np.float32)

        ek_pk = cat([ek[2 * c + e][i * P:(i + 1) * P, :] * SWE
                     for e in range(2) for i in range(DC)])
        ev_pk = cat([ev[2 * c + e][i * P:(i + 1) * P, :] * SV
                     for e in range(2) for i in range(FCH)])
        sk_pk = cat([sk[0][i * P:(i + 1) * P, c * SHF:(c + 1) * SHF] * SWE
                     for i in range(DC)])
        mo8_pack = pack(mo8_l, mo8_c, {
            "ek": ek_pk, "ev": ev_pk, "sk": sk_pk,
        }, F8)
        m16_pack = pack(m16_l, m16_c, {
            "sv": sv[0][c * SHF:(c + 1) * SHF, :] * SVSH,
        }, BF)

        in_maps.append(dict(
            xT=xTp.astype(np.float32),
            atn16=atn_pack, atn8=at8_pack, mid8=mid_pack, moe8=mo8_pack,
            moe16=m16_pack, p32=p32_pack,
        ))
    return in_maps, a1, a2, compact_mask


def kernel(**inputs):
    from concourse import bass_utils

    sim = bool(os.environ.get("BASSK_SIM"))
    sim_gelu = sim or bool(os.environ.get("BASSK_COMPOSED_GELU"))
    rs8 = not bool(os.environ.get("BASSK_NO_RS8"))
    in_maps, a1v, a2v, compact_mask = _prep_inputs(inputs)
    key = (a1v, a2v, sim_gelu, compact_mask, rs8)
    if key not in _PROG_CACHE:
        _PROG_CACHE[key] = _build_program(a1v, a2v, sim_gelu,
                                          compact_mask, rs8)
    nc = _PROG_CACHE[key]

    if sim:
        from concourse.bass_interp import MultiCoreSim

        simu = MultiCoreSim(nc, num_cores=NCORES)
        for c in range(NCORES):
            for k, v in in_maps[c].items():
                simu.cores[c].tensor(k)[:] = v
        simu.simulate(check_with_hw=False)
        shards = [np.array(simu.cores[c].tensor("outT")) for c in range(NCORES)]
    else:
        trace = bool(os.environ.get("BASSK_TRACE"))
        # Optional warm-up execution (measured: does not reduce the
        # inter-core dispatch stagger in this environment; off by default)
        if os.environ.get("BASSK_WARMRUN"):
            bass_utils.run_bass_kernel_spmd(
                nc, in_maps, core_ids=list(range(NCORES)), trace=False
            )
        res = bass_utils.run_bass_kernel_spmd(
            nc, in_maps, core_ids=list(range(NCORES)), trace=trace
        )
        LAST_INFO["exec_time_ns"] = res.exec_time_ns
        LAST_INFO["profile_json"] = res.profile_json
        shards = [np.asarray(res.results[c]["outT"]) for c in range(NCORES)]

    outT = np.concatenate(shards, axis=0)              # [D,S]
    return np.ascontiguousarray(outT.T).reshape(1, S, Dm).astype(np.float32)
